# revision 1
# baseline (speedup 1.0000x reference)
"""Trainium2 Bass kernel for nn_DirectedODRLoss (retrieval_knn).

Math (B=4096, D=256, k=25, scales (1,2,3)):
    dist²(i,j) = |f_i|² + |f_j|² − 2 f_i·f_j ;  y := −dist²  (computed directly
        by an augmented GEMM whose extra contraction rows carry −|f|²)
    topk:  per row, the 25 largest y  (= 25 smallest dist²) via DVE max8 +
        match_replace;  τ_i := 25th largest y;  σ_i = mean(sqrt(−y_clamped+eps))
    mutual knn mask:  y symmetric  ⇒  mutual(i,j) = [y_ij ≥ max(τ_i, τ_j)]
    Wn = exp(y·rσ_i·rσ_j − BIG·(1−mask·dir)),  dir = [lab_i ≤ lab_j]
    S_i = ΣWn + 1,  P = Wn/S + diag(1/S)   (diagonal added by DMA-accumulate)
    loss = (1/B)(C1 + C2/2 + C3/3) with
        C1 = <P, pen>, C2 = <P², pen>, C3 = <P³, pen> = <A, V>,
        A = P² (row shard), V = pen·Pᵀ (row shard), pen_ij = relu(s_i−s_j).

Sharding: rows are split across 8 cores. P is all-gathered (bf16) for the two
B³ GEMMs; Pᵀ is all-gathered from per-core PE-transposed shards. Final scalars
all-reduced. y/W strips are kept in fp16 in SBUF (halves SBUF + doubles DVE).
"""

import numpy as np

import concourse.bacc as bacc
import concourse.bass as bass
import concourse.mybir as mybir
import concourse.tile as tile
from concourse.masks import make_identity

F32 = mybir.dt.float32
F32R = mybir.dt.float32r
F16 = mybir.dt.float16
BF16 = mybir.dt.bfloat16
AX = mybir.AxisListType
OP = mybir.AluOpType
ACT = mybir.ActivationFunctionType

EPS = 1e-8
KNN = 25
BIG = 30000.0
NEG_INF = -1e30


def build_program(B=4096, D=256, NC=8):
    P = 128
    R = B // NC            # rows per core
    NMT = R // P           # row tiles per core
    TN = R                 # column tile (must equal R: Pᵀ AG block alignment)
    assert TN <= 512
    NNT = B // TN          # column tiles
    KC = B // P            # contraction chunks for the B-GEMMs
    GK = D // P            # contraction chunks for the Gram GEMM

    nc = bacc.Bacc("TRN2", target_bir_lowering=False, debug=False,
                   num_devices=NC)

    # ---- I/O ----------------------------------------------------------------
    ft2 = nc.dram_tensor("ft2", [D, R], F32, kind="ExternalInput")     # 2·F_shardᵀ
    ftf = nc.dram_tensor("ftf", [D, B], F32, kind="ExternalInput")     # Fᵀ (full)
    ff = nc.dram_tensor("ff", [B, D], F32, kind="ExternalInput")       # F (full)
    fsh = nc.dram_tensor("fsh", [R, D], F32, kind="ExternalInput")     # F shard
    srow = nc.dram_tensor("srow", [1, B], F32, kind="ExternalInput")   # scores
    smyrow = nc.dram_tensor("smyrow", [1, R], F32, kind="ExternalInput")
    scols = nc.dram_tensor("scols", [P, NMT], F32, kind="ExternalInput")
    lrow = nc.dram_tensor("lrow", [1, B], F32, kind="ExternalInput")
    lcols = nc.dram_tensor("lcols", [P, NMT], F32, kind="ExternalInput")
    loss_out = nc.dram_tensor("loss", [1, 1], F32, kind="ExternalOutput")

    # ---- internal DRAM ------------------------------------------------------
    pn_dram = nc.dram_tensor("pn_dram", [R, B], BF16)
    pt_dram = nc.dram_tensor("pt_dram", [B, R], BF16)                  # Pn_shardᵀ
    pfull = nc.dram_tensor("pfull", [NC * R, B], BF16, addr_space="Shared")
    ptfull = nc.dram_tensor("ptfull", [NC * B, R], BF16, addr_space="Shared")
    stats_in = nc.dram_tensor("stats_in", [1, 2 * R], F32)
    stats_out = nc.dram_tensor("stats_out", [NC, 2 * R], F32, addr_space="Shared")
    k2r_dram = nc.dram_tensor("k2r_dram", [2, B], F32)
    k2l_dram = nc.dram_tensor("k2l_dram", [2, R], F32)
    invs_dram = nc.dram_tensor("invs_dram", [P, NMT], F32)
    red_in = nc.dram_tensor("red_in", [1, 8], F32)
    red_out = nc.dram_tensor("red_out", [1, 8], F32, addr_space="Shared")

    rg = [list(range(NC))]

    with tile.TileContext(nc) as tc:
        with (
            tc.tile_pool(name="const", bufs=1) as constp,
            tc.tile_pool(name="io", bufs=3) as iop,
            tc.tile_pool(name="big", bufs=1) as bigp,
            tc.tile_pool(name="strip", bufs=3) as stripp,
            tc.tile_pool(name="cols", bufs=1) as colp,
            tc.tile_pool(name="work", bufs=2) as workp,
            tc.tile_pool(name="psum", bufs=1, space="PSUM") as psump,
        ):
            def ps_tile(tag, shape=None, dtype=F32):
                return psump.tile(shape or [P, TN], dtype, tag=tag, name=tag)

            # ============ stage A: squared norms ============================
            sqc = colp.tile([P, B // P], F32, tag="sqc")      # |f|² (full, cols)
            sqcs = colp.tile([P, NMT], F32, tag="sqcs")       # |f|² (shard, cols)
            for t in range(B // P):
                ftile = iop.tile([P, D], F32, tag="ftile")
                nc.sync.dma_start(ftile[:], ff[t * P:(t + 1) * P, :])
                scr = workp.tile([P, D], F32, tag="sqscr")
                nc.scalar.activation(scr[:], ftile[:], ACT.Square,
                                     accum_out=sqc[:, t:t + 1])
            for q in range(NMT):
                ftile = iop.tile([P, D], F32, tag="ftile")
                nc.sync.dma_start(ftile[:], fsh[q * P:(q + 1) * P, :])
                scr = workp.tile([P, D], F32, tag="sqscr")
                nc.scalar.activation(scr[:], ftile[:], ACT.Square,
                                     accum_out=sqcs[:, q:q + 1])
            # negate in column layout (round to f32r for the Gram matmul)
            sqcr = colp.tile([P, B // P], F32, tag="sqcr")
            sqcsr = colp.tile([P, NMT], F32, tag="sqcsr")
            nc.vector.tensor_scalar(sqcr[:], sqc[:], -1.0, None, OP.mult)
            nc.vector.tensor_scalar(sqcsr[:], sqcs[:], -1.0, None, OP.mult)

            # k2 (augmentation) operands, padded to 128 partitions.
            # lhsT rows: [1, −|f_m|², 0...];  rhs rows: [−|f_n|², 1, 0...]
            # (partition-offset-1 SBUF writes are not allowed, so assemble the
            #  two rows in DRAM and load them with one base-0 DMA)
            ones_row = constp.tile([1, R], F32, tag="ones_row")
            nc.vector.memset(ones_row[:], 1.0)
            for t in range(B // R):
                nc.sync.dma_start(k2r_dram[1:2, t * R:(t + 1) * R], ones_row[:])
            nc.sync.dma_start(k2l_dram[0:1, :], ones_row[:])
            # row layouts: flat[g] with g = c*128 + p  ⇐  sbuf cols [p, c]
            nc.sync.dma_start(bass.AP(k2r_dram, 0, [[1, P], [P, B // P]]), sqcr[:])
            nc.sync.dma_start(bass.AP(k2l_dram, R, [[1, P], [P, NMT]]), sqcsr[:])
            lhs_k2 = constp.tile([P, R], F32, tag="lhs_k2")
            rhs_k2 = constp.tile([P, TN], F32, tag="rhs_k2")
            nc.vector.memset(lhs_k2[:], 0.0)
            nc.vector.memset(rhs_k2[:], 0.0)
            nc.sync.dma_start(lhs_k2[0:2, :], k2l_dram[:, :])

            # ============ stage A2: Gram → y (fp16 strips) ==================
            ft2_sb = constp.tile([P, GK * R], F32, tag="ft2_sb")
            for g in range(GK):
                nc.sync.dma_start(ft2_sb[:, g * R:(g + 1) * R],
                                  ft2[g * P:(g + 1) * P, :])

            y_all = bigp.tile([P, NMT * B], F32, tag="ybuf")
            for nt in range(NNT):
                gps = [ps_tile(f"pa{m}") for m in range(NMT)]
                for g in range(GK):
                    gt = iop.tile([P, TN], F32, tag="rt", name="gt")
                    nc.sync.dma_start(gt[:], ftf[g * P:(g + 1) * P,
                                                 nt * TN:(nt + 1) * TN])
                    for mt in range(NMT):
                        nc.tensor.matmul(
                            gps[mt][:],
                            ft2_sb[:, g * R + mt * P: g * R + (mt + 1) * P],
                            gt[:],
                            start=(g == 0), stop=False)
                nc.sync.dma_start(rhs_k2[0:2, :],
                                  k2r_dram[:, nt * TN:(nt + 1) * TN])
                for mt in range(NMT):
                    nc.tensor.matmul(
                        gps[mt][:],
                        lhs_k2[:, mt * P:(mt + 1) * P],
                        rhs_k2[:],
                        start=False, stop=True)
                    nc.scalar.activation(
                        y_all[:, mt * B + nt * TN: mt * B + (nt + 1) * TN],
                        gps[mt][:], ACT.Copy)

            # ============ stage B: top-k → τ, rσ ============================
            vals = colp.tile([P, NMT * 32], F32, tag="vals")
            yt_cols = colp.tile([P, NMT], F32, tag="yt_cols")
            rs_cols = colp.tile([P, NMT], F32, tag="rs_cols")
            ssum = colp.tile([P, NMT], F32, tag="ssum")
            eps_c = constp.tile([P, 1], F32, tag="eps_c")
            nc.vector.memset(eps_c[:], EPS)
            for mt in range(NMT):
                ys = y_all[:, mt * B:(mt + 1) * B]
                sa = stripp.tile([P, B], F32, tag="strip", name="sa")
                sb = stripp.tile([P, B], F32, tag="strip", name="sb")
                nc.scalar.activation(sa[:], ys, ACT.Copy)
                v = vals[:, mt * 32:(mt + 1) * 32]
                nc.vector.max(out=v[:, 0:8], in_=sa[:])
                nc.vector.match_replace(out=sb[:], in_to_replace=v[:, 0:8],
                                        in_values=sa[:], imm_value=NEG_INF)
                nc.vector.max(out=v[:, 8:16], in_=sb[:])
                nc.vector.match_replace(out=sa[:], in_to_replace=v[:, 8:16],
                                        in_values=sb[:], imm_value=NEG_INF)
                nc.vector.max(out=v[:, 16:24], in_=sa[:])
                nc.vector.match_replace(out=sb[:], in_to_replace=v[:, 16:24],
                                        in_values=sa[:], imm_value=NEG_INF)
                nc.vector.max(out=v[:, 24:32], in_=sb[:])
                # τ_i = 25th largest y
                nc.vector.tensor_copy(yt_cols[:, mt:mt + 1], v[:, 24:25])
                # σ_i = mean sqrt(max(d,0)+eps) over 25 NN;  d = −y
                c25 = workp.tile([P, KNN], F32, tag="c25")
                nc.vector.tensor_scalar(c25[:], v[:, 0:KNN], 0.0, None, OP.min)
                s25 = workp.tile([P, KNN], F32, tag="s25")
                nc.scalar.activation(s25[:], c25[:], ACT.Sqrt,
                                     bias=eps_c[:, 0:1], scale=-1.0,
                                     accum_out=ssum[:, mt:mt + 1])
            nc.vector.reciprocal(rs_cols[:], ssum[:])
            nc.vector.tensor_scalar(rs_cols[:], rs_cols[:], float(KNN), None,
                                    OP.mult)

            # stats all-gather: flat per-rank [τ(R) ++ rσ(R)], both in
            # shard-row order g_local = c*128 + p  →  AG output is directly
            # the full vector in global row order.
            nc.sync.dma_start(bass.AP(stats_in, 0, [[1, P], [P, NMT]]),
                              yt_cols[:])
            nc.sync.dma_start(bass.AP(stats_in, R, [[1, P], [P, NMT]]),
                              rs_cols[:])
            nc.gpsimd.collective_compute(
                "AllGather", OP.bypass, replica_groups=rg,
                ins=[stats_in.ap().opt()], outs=[stats_out.ap().opt()])

            def stat_bcast_ap(off):
                return bass.AP(stats_out, off, [[0, P], [2 * R, NC], [1, R]])

            yt_b = stripp.tile([P, B], F32, tag="strip", name="yt_b")
            rs_b = stripp.tile([P, B], F32, tag="strip", name="rs_b")
            lab_b = stripp.tile([P, B], F32, tag="strip", name="lab_b")
            nc.sync.dma_start(yt_b[:].rearrange("a (r q) -> a r q", r=NC),
                              stat_bcast_ap(0))
            nc.sync.dma_start(rs_b[:].rearrange("a (r q) -> a r q", r=NC),
                              stat_bcast_ap(R))
            nc.sync.dma_start(lab_b[:], bass.AP(lrow, 0, [[0, P], [1, B]]))

            lab_c = colp.tile([P, NMT], F32, tag="lab_c")
            s_c = colp.tile([P, NMT], F32, tag="s_c")
            nc.sync.dma_start(lab_c[:], lcols[:, :])
            nc.sync.dma_start(s_c[:], scols[:, :])

            # ============ stage W: Wn, S, Pn, C1 ============================
            srcols = colp.tile([P, NMT * NNT], F32, tag="srcols")
            c1cols = colp.tile([P, NMT * NNT], F32, tag="c1cols")
            s_b = constp.tile([P, B], F32, tag="s_b")
            nc.sync.dma_start(s_b[:], bass.AP(srow, 0, [[0, P], [1, B]]))

            for mt in range(NMT):
                for nt in range(NNT):
                    ys = y_all[:, mt * B + nt * TN: mt * B + (nt + 1) * TN]
                    thr = workp.tile([P, TN], F32, tag="w1", name="thr")
                    nc.vector.tensor_scalar(thr[:], yt_b[:, nt * TN:(nt + 1) * TN],
                                            yt_cols[:, mt:mt + 1], None, OP.max)
                    keep = workp.tile([P, TN], F32, tag="w2", name="keep")
                    nc.vector.tensor_tensor(keep[:], ys, thr[:], OP.is_ge)
                    dirk = workp.tile([P, TN], F32, tag="w3", name="dirk")
                    nc.vector.tensor_scalar(dirk[:], lab_b[:, nt * TN:(nt + 1) * TN],
                                            lab_c[:, mt:mt + 1], None, OP.is_ge)
                    mask = workp.tile([P, TN], F32, tag="w4", name="mask")
                    nc.vector.tensor_tensor(mask[:], keep[:], dirk[:], OP.mult)
                    # 1 → 0,  0 → −BIG
                    nc.vector.tensor_scalar(mask[:], mask[:], BIG, -BIG,
                                            OP.mult, op1=OP.add)
                    e = workp.tile([P, TN], F32, tag="w1", name="e")
                    nc.vector.tensor_tensor(e[:], ys, rs_b[:, nt * TN:(nt + 1) * TN],
                                            OP.mult)
                    nc.vector.tensor_tensor(e[:], e[:], mask[:], OP.add)
                    # Wn = exp(e·rσ_i), in place over y
                    nc.scalar.activation(ys, e[:], ACT.Exp,
                                         scale=rs_cols[:, mt:mt + 1],
                                         accum_out=srcols[:, mt * NNT + nt:
                                                          mt * NNT + nt + 1])
                    # C1 partial: Σ Wn·pen (row scale by 1/S applied later)
                    pen = workp.tile([P, TN], F32, tag="w2", name="pen")
                    nc.scalar.activation(pen[:], s_b[:, nt * TN:(nt + 1) * TN],
                                         ACT.Relu, bias=s_c[:, mt:mt + 1],
                                         scale=-1.0)
                    prod = workp.tile([P, TN], F32, tag="w3", name="prod")
                    nc.gpsimd.tensor_tensor(prod[:], ys, pen[:], OP.mult)
                    junk = workp.tile([P, TN], F32, tag="w1", name="junk")
                    nc.scalar.activation(junk[:], prod[:], ACT.Copy,
                                         accum_out=c1cols[:, mt * NNT + nt:
                                                          mt * NNT + nt + 1])

            # S = ΣWn + 1 ;  invS = 1/S
            invS = colp.tile([P, NMT], F32, tag="invS")
            Scol = colp.tile([P, NMT], F32, tag="Scol")
            for mt in range(NMT):
                nc.vector.reduce_sum(Scol[:, mt:mt + 1],
                                     srcols[:, mt * NNT:(mt + 1) * NNT], axis=AX.X)
            nc.vector.tensor_scalar(Scol[:], Scol[:], 1.0, None, OP.add)
            nc.vector.reciprocal(invS[:], Scol[:])
            nc.sync.dma_start(invs_dram[:, :], invS[:])

            # Pn tiles (bf16) → DRAM
            for mt in range(NMT):
                for nt in range(NNT):
                    pn_t = workp.tile([P, TN], BF16, tag="pn_t")
                    nc.vector.tensor_scalar(
                        pn_t[:], y_all[:, mt * B + nt * TN: mt * B + (nt + 1) * TN],
                        invS[:, mt:mt + 1], None, OP.mult)
                    nc.sync.dma_start(pn_dram[mt * P:(mt + 1) * P,
                                              nt * TN:(nt + 1) * TN], pn_t[:])

            # C1 finalize (per-row 1/S)
            c1v = colp.tile([P, 1], F32, tag="c1v")
            c1r = colp.tile([P, NMT], F32, tag="c1r")
            for mt in range(NMT):
                nc.vector.reduce_sum(c1r[:, mt:mt + 1],
                                     c1cols[:, mt * NNT:(mt + 1) * NNT], axis=AX.X)
            nc.vector.tensor_tensor(c1r[:], c1r[:], invS[:], OP.mult)
            nc.vector.reduce_sum(c1v[:], c1r[:], axis=AX.X)

            # ============ diagonal fix: P += diag(1/S) ======================
            invs_rowf = colp.tile([1, R], F32, tag="invs_rowf")
            nc.sync.dma_start(invs_rowf[:].rearrange("a (c p) -> a c p", p=P),
                              bass.AP(invs_dram, 0, [[0, 1], [1, NMT], [NMT, P]]))
            invs_row = colp.tile([1, R], BF16, tag="invs_row")
            nc.vector.tensor_copy(invs_row[:], invs_rowf[:])
            rank = nc.gpsimd.partition_id()
            diag_ap = pn_dram.ap().rearrange("a b -> () (a b)")[
                0:1, bass.ds(rank * R, R, B + 1)]
            nc.gpsimd.dma_start(diag_ap, invs_row[0:1, :], accum_op=OP.add)

            # ============ transposes → lhsT (and Pᵀ AG input) ===============
            ident = constp.tile([P, P], BF16, tag="ident")
            make_identity(nc, ident[:])
            lp_buf = bigp.tile([P, 2 * KC * R], BF16, tag="ybuf", name="lp_buf")
            lhsT = lp_buf[:, 0:KC * R]
            for q in range(NMT):
                for kb in range(KC):
                    src = workp.tile([P, P], BF16, tag="tsrc")
                    nc.sync.dma_start(src[:], pn_dram[q * P:(q + 1) * P,
                                                      kb * P:(kb + 1) * P])
                    pst = ps_tile(f"pv{kb % 4}", shape=[P, P], dtype=BF16)
                    nc.tensor.transpose(pst[:], src[:], ident[:])
                    nc.any.tensor_copy(
                        lhsT[:, kb * R + q * P: kb * R + (q + 1) * P], pst[:])
            # write Pnᵀ shard for the Pᵀ all-gather
            for kb in range(KC):
                nc.sync.dma_start(pt_dram[kb * P:(kb + 1) * P, :],
                                  lhsT[:, kb * R:(kb + 1) * R])

            # ============ all-gathers ======================================
            nc.gpsimd.collective_compute(
                "AllGather", OP.bypass, replica_groups=rg,
                ins=[pn_dram.ap().opt()], outs=[pfull.ap().opt()])
            nc.gpsimd.collective_compute(
                "AllGather", OP.bypass, replica_groups=rg,
                ins=[pt_dram.ap().opt()], outs=[ptfull.ap().opt()])

            # ============ penᵀ chunks (lhsT of the V-GEMM) ==================
            smy_b = constp.tile([P, R], F32, tag="smy_b")
            nc.sync.dma_start(smy_b[:], bass.AP(smyrow, 0, [[0, P], [1, R]]))
            sfc = colp.tile([P, KC], F32, tag="sfc")       # −s_j, col layout
            nc.sync.dma_start(sfc[:], bass.AP(srow, 0, [[1, P], [P, KC]]))
            nc.vector.tensor_scalar(sfc[:], sfc[:], -1.0, None, OP.mult)
            penT = lp_buf[:, KC * R:2 * KC * R]
            for kb in range(KC):
                nc.scalar.activation(penT[:, kb * R:(kb + 1) * R], smy_b[:],
                                     ACT.Relu, bias=sfc[:, kb:kb + 1], scale=1.0)

            # ============ main GEMMs + contractions =========================
            c2cols = colp.tile([P, NMT * NNT], F32, tag="c2cols")
            c3cols = colp.tile([P, NMT * NNT], F32, tag="c3cols")
            for nt in range(NNT):
                pa = [ps_tile(f"pa{m}") for m in range(NMT)]
                pv = [ps_tile(f"pv{m}") for m in range(NMT)]
                for kb in range(KC):
                    rt = iop.tile([P, TN], BF16, tag="rt", name="rt")
                    nc.sync.dma_start(rt[:], pfull[kb * P:(kb + 1) * P,
                                                   nt * TN:(nt + 1) * TN])
                    for m in range(NMT):
                        nc.tensor.matmul(pa[m][:],
                                         lhsT[:, kb * R + m * P: kb * R + (m + 1) * P],
                                         rt[:], start=(kb == 0), stop=(kb == KC - 1))
                    rtv = iop.tile([P, TN], BF16, tag="rtv", name="rtv")
                    nc.sync.dma_start(rtv[:], ptfull[nt * B + kb * P:
                                                     nt * B + (kb + 1) * P, :])
                    for m in range(NMT):
                        nc.tensor.matmul(pv[m][:],
                                         penT[:, kb * R + m * P: kb * R + (m + 1) * P],
                                         rtv[:], start=(kb == 0), stop=(kb == KC - 1))
                for m in range(NMT):
                    zs = workp.tile([P, TN], F32, tag="w4", name="zs")
                    nc.scalar.activation(zs[:], pv[m][:], ACT.Copy)
                    pen = workp.tile([P, TN], F32, tag="w2", name="pen")
                    nc.scalar.activation(pen[:], s_b[:, nt * TN:(nt + 1) * TN],
                                         ACT.Relu, bias=s_c[:, m:m + 1], scale=-1.0)
                    prodA = workp.tile([P, TN], F32, tag="w3", name="prodA")
                    nc.vector.tensor_tensor(prodA[:], pa[m][:], pen[:], OP.mult)
                    junk = workp.tile([P, TN], F32, tag="w1", name="junk")
                    nc.scalar.activation(junk[:], prodA[:], ACT.Copy,
                                         accum_out=c2cols[:, nt * NMT + m:
                                                          nt * NMT + m + 1])
                    prodZ = workp.tile([P, TN], F32, tag="w3", name="prodZ")
                    nc.vector.tensor_tensor(prodZ[:], pa[m][:], zs[:], OP.mult)
                    junk2 = workp.tile([P, TN], F32, tag="w1", name="junk2")
                    nc.scalar.activation(junk2[:], prodZ[:], ACT.Copy,
                                         accum_out=c3cols[:, nt * NMT + m:
                                                          nt * NMT + m + 1])

            # ============ final reduction ==================================
            c2v = colp.tile([P, 1], F32, tag="c2v")
            c3v = colp.tile([P, 1], F32, tag="c3v")
            nc.vector.reduce_sum(c2v[:], c2cols[:], axis=AX.X)
            nc.vector.reduce_sum(c3v[:], c3cols[:], axis=AX.X)
            tot = colp.tile([P, 1], F32, tag="tot")
            nc.vector.tensor_scalar(tot[:], c2v[:], 0.5, None, OP.mult)
            nc.vector.tensor_tensor(tot[:], tot[:], c1v[:], OP.add)
            nc.vector.tensor_scalar(c3v[:], c3v[:], 1.0 / 3.0, None, OP.mult)
            nc.vector.tensor_tensor(tot[:], tot[:], c3v[:], OP.add)

            ones_c = constp.tile([P, 1], F32, tag="ones_c")
            nc.vector.memset(ones_c[:], 1.0)
            fin = ps_tile("pa0", shape=[1, 8])
            nc.tensor.matmul(fin[:, 0:1], tot[:], ones_c[:], start=True, stop=True)
            lsb = colp.tile([1, 8], F32, tag="lsb")
            nc.vector.memset(lsb[:], 0.0)
            nc.scalar.activation(lsb[:, 0:1], fin[:, 0:1], ACT.Copy,
                                 scale=1.0 / float(B))
            nc.sync.dma_start(red_in[:, :], lsb[:])
            nc.gpsimd.collective_compute(
                "AllReduce", OP.add, replica_groups=rg,
                ins=[red_in.ap().opt()], outs=[red_out.ap().opt()])
            nc.sync.dma_start(loss_out[:, :], red_out[0:1, 0:1])

    nc.compile()
    return nc


def make_inputs(features, scores, labels, B, D, NC):
    """Build the per-core input maps from full inputs."""
    R = B // NC
    P = 128
    NMT = R // P
    f = np.ascontiguousarray(features, dtype=np.float32)
    s = np.ascontiguousarray(scores, dtype=np.float32).reshape(B)
    lab = np.asarray(labels).astype(np.float32).reshape(B)
    ftf = np.ascontiguousarray(f.T)
    in_maps = []
    for c in range(NC):
        sh = slice(c * R, (c + 1) * R)
        in_maps.append({
            "ft2": np.ascontiguousarray(2.0 * f[sh].T),
            "ftf": ftf,
            "ff": f,
            "fsh": np.ascontiguousarray(f[sh]),
            "srow": s.reshape(1, B),
            "smyrow": np.ascontiguousarray(s[sh]).reshape(1, R),
            "scols": np.ascontiguousarray(s[sh].reshape(NMT, P).T),
            "lrow": lab.reshape(1, B),
            "lcols": np.ascontiguousarray(lab[sh].reshape(NMT, P).T),
        })
    return in_maps


_cached = {}


def kernel(features, scores, labels):
    B, D = features.shape
    NC = 8
    key = (B, D)
    if key not in _cached:
        _cached[key] = build_program(B=B, D=D, NC=NC)
    nc = _cached[key]
    from concourse.bass_utils import run_bass_kernel_spmd
    in_maps = make_inputs(features, scores, labels, B, D, NC)
    res = run_bass_kernel_spmd(nc, in_maps, core_ids=list(range(NC)))
    out = res.results[0]["loss"]
    return np.float32(out.reshape(())[()])



# revision 30
# speedup vs baseline: 1.8832x; 1.8832x over previous
"""Trainium2 Bass kernel for nn_DirectedODRLoss (retrieval_knn).

Math (B=4096, D=256, k=25, scales (1,2,3)):
    dist²(i,j) = |f_i|² + |f_j|² − 2 f_i·f_j ;  y := −dist²  (f32 strips;
        bf16 Gram GEMM; −|f_i|² per-partition scalar and −|f_j|² f32 row
        strip folded into one DVE scalar_tensor_tensor on the PSUM drain)
    topk: per row, candidates = top-8 of each of 16 256-wide chunks (16
        f32 max8 passes) → top-25 of the 128 candidates via max8 +
        match_replace;  τ_i := 25th largest y;  σ_i = mean(sqrt(−y+eps))
    mutual knn mask:  y symmetric ⇒ mutual(i,j) = [y_ij ≥ max(τ_i, τ_j)]
    drop = [max(τ_i,τ_j) > y] | [lab_j < lab_i];  e = y·rσ_j (fp16)
    Wn = exp(rσ_i·(e − 1024·drop))   (drop ⇒ exp(≤ −45/σ²) ⇒ 0 in fp16)
    S_i = ΣWn + 1,  P = (Wn + dsel·I)/S  (f32 in place; diag via a
        per-core one-hot dsel over column blocks), quantized to fp8 e4m3
    loss = (1/B)(C1 + C2/2 + C3/3) with
        C1 = <Wn/S, pen>,  C2 = <A, pen>,  C3 = <U, pen>,
        A = P²  (computed transposed: Aᵀ chunks, lhsT = raw pfull column
        strips, rhs = SBUF-resident P_shardᵀ),  U = A·P (Uᵀ likewise),
        pen_ij = relu(s_i − s_j) (fp16).

Sharding: rows split across 8 cores. P all-gathered in fp8; both B³ GEMMs
run in fp8 DoubleRow mode (0.5 cyc/row); Aᵀ/Uᵀ chunks consumed against
recomputed penᵀ tiles via fused scalar_tensor_tensor accumulations.
Final scalars all-reduced.
"""

import ml_dtypes
import numpy as np

import concourse.bacc as bacc
import concourse.bass as bass
import concourse.mybir as mybir
import concourse.tile as tile
from concourse.masks import make_identity

F32 = mybir.dt.float32
F16 = mybir.dt.float16
BF16 = mybir.dt.bfloat16
F8 = mybir.dt.float8e4
AX = mybir.AxisListType
OP = mybir.AluOpType
ACT = mybir.ActivationFunctionType
DRM = mybir.MatmulPerfMode.DoubleRow

EPS = 1e-8
KNN = 25
DROP_SHIFT = 1024.0
NEG_FILL = -1e30


def build_program(B=4096, D=256, NC=8):
    P = 128
    R = B // NC            # rows per core (512)
    NMT = R // P           # row tiles per core (4)
    TN = R                 # column tile (512)
    NNT = B // TN          # column tiles (8)
    KC = B // P            # 128-row chunks of B (32)
    KC2 = KC // 2          # DoubleRow steps (16)
    GK = D // P            # contraction chunks for the Gram GEMM (2)
    TC = 16                # topk candidate chunks per row
    CW = B // TC           # chunk width (256)

    nc = bacc.Bacc("TRN2", target_bir_lowering=False, debug=False,
                   num_devices=NC)

    # ---- I/O ----------------------------------------------------------------
    ft2 = nc.dram_tensor("ft2", [D, R], BF16, kind="ExternalInput")    # 2·F_shardᵀ
    ftf = nc.dram_tensor("ftf", [D, B], BF16, kind="ExternalInput")    # Fᵀ (full)
    fsh = nc.dram_tensor("fsh", [R, D], BF16, kind="ExternalInput")    # F shard
    srow = nc.dram_tensor("srow", [1, B], F32, kind="ExternalInput")   # scores
    srow16 = nc.dram_tensor("srow16", [1, B], F16, kind="ExternalInput")
    smyrow16 = nc.dram_tensor("smyrow16", [1, R], F16, kind="ExternalInput")
    scols = nc.dram_tensor("scols", [P, NMT], F32, kind="ExternalInput")
    lrow16 = nc.dram_tensor("lrow16", [1, B], F16, kind="ExternalInput")
    lcols = nc.dram_tensor("lcols", [P, NMT], F32, kind="ExternalInput")
    dsel = nc.dram_tensor("dsel", [1, NNT], F32, kind="ExternalInput")  # one-hot(rank)
    loss_out = nc.dram_tensor("loss", [1, 1], F32, kind="ExternalOutput")

    # ---- internal DRAM ------------------------------------------------------
    pn_dram = nc.dram_tensor("pn_dram", [R, B], F8)
    pfull = nc.dram_tensor("pfull", [NC * R, B], F8, addr_space="Shared")
    stats_in = nc.dram_tensor("stats_in", [1, 2 * R], F32)
    stats_out = nc.dram_tensor("stats_out", [NC, 2 * R], F32, addr_space="Shared")
    d_dram = nc.dram_tensor("d_dram", [1, R], F32)
    d_out = nc.dram_tensor("d_out", [NC, R], F32, addr_space="Shared")
    red_in = nc.dram_tensor("red_in", [1, 8], F32)
    red_out = nc.dram_tensor("red_out", [1, 8], F32, addr_space="Shared")

    rg = [list(range(NC))]

    with tile.TileContext(nc) as tc:
        with (
            tc.tile_pool(name="const", bufs=1) as constp,
            tc.tile_pool(name="io", bufs=2) as iop,
            tc.tile_pool(name="pnp", bufs=1) as pnp,
            tc.tile_pool(name="big", bufs=1) as bigp,
            tc.tile_pool(name="strip", bufs=1) as stripp,
            tc.tile_pool(name="cols", bufs=1) as colp,
            tc.tile_pool(name="work", bufs=2) as workp,
            tc.tile_pool(name="psum", bufs=1, space="PSUM") as psump,
        ):
            def ps_tile(tag, shape=None, dtype=F32):
                return psump.tile(shape or [P, TN], dtype, tag=tag, name=tag)

            # Gram operands (bf16, cached in SBUF, reused per mt tile)
            ft2_sb = constp.tile([P, GK * R], BF16, tag="ft2_sb")
            for g in range(GK):
                nc.sync.dma_start(ft2_sb[:, g * R:(g + 1) * R],
                                  ft2[g * P:(g + 1) * P, :])
            ftf_sb = bigp.tile([P, GK * B], BF16, tag="fbuf", name="ftf_sb")
            for g in range(GK):
                nc.sync.dma_start(ftf_sb[:, g * B:(g + 1) * B],
                                  ftf[g * P:(g + 1) * P, :])

            # ============ stage A: squared norms ============================
            # full |f|²: square ftf (f32 out), column-reduce via ones-matmul
            # broadcast to all 128 partitions (lhsT = all-ones matrix)
            sq2 = bigp.tile([P, GK * B], F32, tag="ybuf", name="sq2")
            for g in range(GK):
                nc.scalar.activation(sq2[:, g * B:(g + 1) * B],
                                     ftf_sb[:, g * B:(g + 1) * B], ACT.Square)
            ones_m = constp.tile([P, P], F32, tag="ones_m")
            nc.vector.memset(ones_m[:], 1.0)
            sqj_b = stripp.tile([P, B], F32, tag="sqj_b")
            for ntb in range(NNT):
                psq = ps_tile(f"pa{ntb % 4}")
                for g in range(GK):
                    nc.tensor.matmul(
                        psq[:], ones_m[:],
                        sq2[:, g * B + ntb * TN: g * B + (ntb + 1) * TN],
                        start=(g == 0), stop=(g == GK - 1))
                nc.scalar.activation(sqj_b[:, ntb * TN:(ntb + 1) * TN],
                                     psq[:], ACT.Copy)
            # shard |f|² in column layout (per-partition scalars, negated)
            sqcs = colp.tile([P, NMT], F32, tag="sqcs")
            fshb = iop.tile([P, NMT * D], BF16, tag="fblk", name="fshb")
            nc.sync.dma_start(
                fshb[:].rearrange("p (t d) -> p t d", t=NMT),
                bass.AP(fsh, 0, [[D, P], [P * D, NMT], [1, D]]))
            for q in range(NMT):
                scr = workp.tile([P, D], F32, tag="sqscr")
                nc.scalar.activation(scr[:], fshb[:, q * D:(q + 1) * D],
                                     ACT.Square, accum_out=sqcs[:, q:q + 1])
            sqcsr = colp.tile([P, NMT], F32, tag="sqcsr")
            nc.vector.tensor_scalar(sqcsr[:], sqcs[:], -1.0, None, OP.mult)

            # misc broadcast loads
            dsel_c = colp.tile([P, NNT], F32, tag="dsel_c")
            nc.sync.dma_start(dsel_c[:], bass.AP(dsel, 0, [[0, P], [1, NNT]]))
            lab_c = colp.tile([P, NMT], F32, tag="lab_c")
            s_c = colp.tile([P, NMT], F32, tag="s_c")
            nc.sync.dma_start(lab_c[:], lcols[:, :])
            nc.sync.dma_start(s_c[:], scols[:, :])
            lab_b = stripp.tile([P, B], F16, tag="lab_b")
            s_b = stripp.tile([P, B], F16, tag="s_b")
            smy_b = constp.tile([P, R], F16, tag="smy_b")
            nc.sync.dma_start(lab_b[:], bass.AP(lrow16, 0, [[0, P], [1, B]]))
            nc.sync.dma_start(s_b[:], bass.AP(srow16, 0, [[0, P], [1, B]]))
            nc.sync.dma_start(smy_b[:], bass.AP(smyrow16, 0, [[0, P], [1, R]]))
            sfc = colp.tile([P, KC], F32, tag="sfc")       # −s_g, col layout
            nc.sync.dma_start(sfc[:], bass.AP(srow, 0, [[1, P], [P, KC]]))
            nc.vector.tensor_scalar(sfc[:], sfc[:], -1.0, None, OP.mult)

            ident16 = constp.tile([P, P], F16, tag="ident16")
            make_identity(nc, ident16[:])
            ndsel_c = colp.tile([P, NNT], F32, tag="ndsel_c")
            nc.vector.tensor_scalar(ndsel_c[:], dsel_c[:], -1.0, None, OP.mult)
            eps_c = constp.tile([P, 1], F32, tag="eps_c")
            nc.vector.memset(eps_c[:], EPS)

            # ============ stage B: Gram → y (f32) + chunked topk ============
            y_all = bigp.tile([P, NMT * B], F32, tag="ybuf", name="y_all")
            cands = colp.tile([P, TC * 8], F32, tag="cands")
            vals = colp.tile([P, 32], F32, tag="vals")
            yt_cols = colp.tile([P, NMT], F32, tag="yt_cols")
            rs_cols = colp.tile([P, NMT], F32, tag="rs_cols")
            ssum = colp.tile([P, NMT], F32, tag="ssum")
            for mt in range(NMT):
                gps = [ps_tile(f"pa{ntb}") for ntb in range(NNT)]
                for g in range(GK):
                    for ntb in range(NNT):
                        nc.tensor.matmul(
                            gps[ntb][:],
                            ft2_sb[:, g * R + mt * P: g * R + (mt + 1) * P],
                            ftf_sb[:, g * B + ntb * TN: g * B + (ntb + 1) * TN],
                            start=(g == 0), stop=(g == GK - 1))
                ys = y_all[:, mt * B:(mt + 1) * B]
                for ntb in range(NNT):
                    # y = (gram − |f_i|²) − |f_j|²   (f32 out)
                    nc.vector.scalar_tensor_tensor(
                        ys[:, ntb * TN:(ntb + 1) * TN], gps[ntb][:],
                        sqcsr[:, mt:mt + 1], sqj_b[:, ntb * TN:(ntb + 1) * TN],
                        op0=OP.add, op1=OP.subtract)
                # candidates: top-8 of each 256-wide chunk
                for t in range(TC):
                    nc.vector.max(out=cands[:, t * 8:(t + 1) * 8],
                                  in_=ys[:, t * CW:(t + 1) * CW])
                # top-25 of the 128 candidates
                ca = workp.tile([P, TC * 8], F32, tag="ca", name="ca")
                cb = workp.tile([P, TC * 8], F32, tag="cb", name="cb")
                nc.vector.max(out=vals[:, 0:8], in_=cands[:])
                nc.vector.match_replace(out=ca[:], in_to_replace=vals[:, 0:8],
                                        in_values=cands[:], imm_value=NEG_FILL)
                nc.vector.max(out=vals[:, 8:16], in_=ca[:])
                nc.vector.match_replace(out=cb[:], in_to_replace=vals[:, 8:16],
                                        in_values=ca[:], imm_value=NEG_FILL)
                nc.vector.max(out=vals[:, 16:24], in_=cb[:])
                nc.vector.match_replace(out=ca[:], in_to_replace=vals[:, 16:24],
                                        in_values=cb[:], imm_value=NEG_FILL)
                nc.vector.max(out=vals[:, 24:32], in_=ca[:])
                # τ_i = 25th largest y
                nc.vector.tensor_copy(yt_cols[:, mt:mt + 1], vals[:, 24:25])
                # σ_i = mean sqrt(max(d,0)+eps) over 25 NN;  d = −y
                c25 = workp.tile([P, KNN], F32, tag="c25")
                nc.vector.tensor_scalar(c25[:], vals[:, 0:KNN], 0.0, None,
                                        OP.min)
                s25 = workp.tile([P, KNN], F32, tag="s25")
                nc.scalar.activation(s25[:], c25[:], ACT.Sqrt,
                                     bias=eps_c[:, 0:1], scale=-1.0,
                                     accum_out=ssum[:, mt:mt + 1])
            nc.vector.reciprocal(rs_cols[:], ssum[:])
            nc.vector.tensor_scalar(rs_cols[:], rs_cols[:], float(KNN), None,
                                    OP.mult)

            # stats all-gather: flat per-rank [τ(R) ++ rσ(R)] (f32), both in
            # shard-row order g_local = c*128 + p.
            nc.sync.dma_start(bass.AP(stats_in, 0, [[1, P], [P, NMT]]),
                              yt_cols[:])
            nc.sync.dma_start(bass.AP(stats_in, R, [[1, P], [P, NMT]]),
                              rs_cols[:])
            nc.gpsimd.collective_compute(
                "AllGather", OP.bypass, replica_groups=rg,
                ins=[stats_in.ap().opt()], outs=[stats_out.ap().opt()])

            def stat_bcast_ap(off):
                return bass.AP(stats_out, off, [[0, P], [2 * R, NC], [1, R]])

            yt_b = stripp.tile([P, B], F32, tag="yt_b")
            nc.sync.dma_start(yt_b[:].rearrange("a (r q) -> a r q", r=NC),
                              stat_bcast_ap(0))
            # rσ strip: stage f32 into sqj_b's buffer (dead after stage B),
            # convert to fp16
            rs_f32 = stripp.tile([P, B], F32, tag="sqj_b", name="rs_f32")
            nc.sync.dma_start(rs_f32[:].rearrange("a (r q) -> a r q", r=NC),
                              stat_bcast_ap(R))
            rs_b = stripp.tile([P, B], F16, tag="rs_b")
            nc.vector.tensor_copy(rs_b[:], rs_f32[:])

            # ============ stage W: Wn, S, P, C1 =============================
            srcols = colp.tile([P, NMT * NNT], F32, tag="srcols")
            c1cols = colp.tile([P, NMT * NNT], F32, tag="c1cols")
            invS = colp.tile([P, NMT], F32, tag="invS")
            Scol = colp.tile([P, NMT], F32, tag="Scol")
            pn8 = [pnp.tile([P, B], F8, tag="pn8", name=f"pn8_{mt}")
                   for mt in range(NMT)]
            for mt in range(NMT):
                ys = y_all[:, mt * B:(mt + 1) * B]
                for ntb in range(NNT):
                    yt_t = ys[:, ntb * TN:(ntb + 1) * TN]
                    # drop = [max(τ_j, τ_i) > y] | [lab_j < lab_i]
                    drop1 = workp.tile([P, TN], F16, tag="w1", name="drop1")
                    nc.vector.scalar_tensor_tensor(
                        drop1[:], yt_b[:, ntb * TN:(ntb + 1) * TN],
                        yt_cols[:, mt:mt + 1], yt_t,
                        op0=OP.max, op1=OP.is_gt)
                    drop = workp.tile([P, TN], F16, tag="w2", name="drop")
                    nc.vector.scalar_tensor_tensor(
                        drop[:], lab_b[:, ntb * TN:(ntb + 1) * TN],
                        lab_c[:, mt:mt + 1], drop1[:],
                        op0=OP.is_lt, op1=OP.max)
                    # e = y·rσ_j (fp16); e2 = e − 1024·drop
                    e = workp.tile([P, TN], F16, tag="w3", name="e")
                    nc.vector.tensor_tensor(
                        e[:], yt_t, rs_b[:, ntb * TN:(ntb + 1) * TN], OP.mult)
                    e2 = workp.tile([P, TN], F16, tag="w1", name="e2")
                    nc.vector.scalar_tensor_tensor(
                        e2[:], drop[:], -DROP_SHIFT, e[:],
                        op0=OP.mult, op1=OP.add)
                    # Wn = exp(rσ_i·e2) (fp16 work tile), accum → ΣWn
                    wn = workp.tile([P, TN], F16, tag="w2", name="wn")
                    nc.scalar.activation(wn[:], e2[:], ACT.Exp,
                                         scale=rs_cols[:, mt:mt + 1],
                                         accum_out=srcols[:, mt * NNT + ntb:
                                                          mt * NNT + ntb + 1])
                    # C1 partial: Σ Wn·pen  (row scale 1/S later)
                    pen = workp.tile([P, TN], F16, tag="w3", name="pen")
                    nc.scalar.activation(pen[:], s_b[:, ntb * TN:(ntb + 1) * TN],
                                         ACT.Relu, bias=s_c[:, mt:mt + 1],
                                         scale=-1.0)
                    junk = workp.tile([P, TN], F16, tag="w1", name="junk")
                    nc.vector.scalar_tensor_tensor(
                        junk[:], wn[:], 1.0, pen[:], op0=OP.mult, op1=OP.mult,
                        accum_out=c1cols[:, mt * NNT + ntb:
                                         mt * NNT + ntb + 1])
                    # store Wn back into the f32 y strip (in place)
                    nc.vector.tensor_copy(yt_t, wn[:])
                # S = ΣWn + 1 ;  invS = 1/S
                nc.vector.reduce_sum(Scol[:, mt:mt + 1],
                                     srcols[:, mt * NNT:(mt + 1) * NNT],
                                     axis=AX.X)
                nc.vector.tensor_scalar(Scol[:, mt:mt + 1], Scol[:, mt:mt + 1],
                                        1.0, None, OP.add)
                nc.vector.reciprocal(invS[:, mt:mt + 1], Scol[:, mt:mt + 1])
                # Q = offdiag(P) = (Wn − dsel·I)/S  (diagonal handled exactly
                # via d = 2/S in the GEMM correction terms)
                for ntb in range(NNT):
                    dslice = ys[:, ntb * TN + mt * P: ntb * TN + (mt + 1) * P]
                    nc.vector.scalar_tensor_tensor(
                        dslice, ident16[:], ndsel_c[:, ntb:ntb + 1], dslice,
                        op0=OP.mult, op1=OP.add)
                    nc.vector.tensor_scalar(ys[:, ntb * TN:(ntb + 1) * TN],
                                            ys[:, ntb * TN:(ntb + 1) * TN],
                                            invS[:, mt:mt + 1], None, OP.mult)
                nc.scalar.activation(pn8[mt][:], ys, ACT.Copy)
                nc.sync.dma_start(
                    bass.AP(pn_dram, mt * P * B, [[B, P], [1, B]]), pn8[mt][:])

            # C1 finalize (per-row 1/S)
            c1v = colp.tile([P, 1], F32, tag="c1v")
            c1r = colp.tile([P, NMT], F32, tag="c1r")
            for mt in range(NMT):
                nc.vector.reduce_sum(c1r[:, mt:mt + 1],
                                     c1cols[:, mt * NNT:(mt + 1) * NNT],
                                     axis=AX.X)
            nc.vector.tensor_tensor(c1r[:], c1r[:], invS[:], OP.mult)
            nc.vector.reduce_sum(c1v[:], c1r[:], axis=AX.X)

            # ============ all-gathers: diag d = 2/S (f32) and Q8 ============
            dloc = colp.tile([P, NMT], F32, tag="dloc")
            nc.vector.tensor_scalar(dloc[:], invS[:], 2.0, None, OP.mult)
            nc.sync.dma_start(bass.AP(d_dram, 0, [[1, P], [P, NMT]]), dloc[:])
            nc.gpsimd.collective_compute(
                "AllGather", OP.bypass, replica_groups=rg,
                ins=[d_dram.ap().opt()], outs=[d_out.ap().opt()])
            nc.gpsimd.collective_compute(
                "AllGather", OP.bypass, replica_groups=rg,
                ins=[pn_dram.ap().opt()], outs=[pfull.ap().opt()])
            # d in column layout over full B, d² gated by the diag one-hot,
            # and shard d broadcast along the free axis
            d_cols = colp.tile([P, KC], F32, tag="d_cols")
            nc.sync.dma_start(d_cols[:], bass.AP(d_out, 0, [[1, P], [P, KC]]))
            d2g = colp.tile([P, KC], F32, tag="d2g")
            nc.vector.tensor_tensor(d2g[:], d_cols[:], d_cols[:], OP.mult)
            for ntb in range(NNT):
                nc.vector.tensor_scalar(
                    d2g[:, ntb * NMT:(ntb + 1) * NMT],
                    d2g[:, ntb * NMT:(ntb + 1) * NMT],
                    dsel_c[:, ntb:ntb + 1], None, OP.mult)
            d_strip = constp.tile([P, R], F32, tag="d_strip")
            nc.sync.dma_start(d_strip[:], bass.AP(d_dram, 0, [[0, P], [1, R]]))

            # ============ P_shardᵀ (fp8, SBUF) via PE transposes ============
            # transpose the f32 P strips (2 cyc/row), cast fp8 on psum drain
            idf32 = constp.tile([P, P], F32, tag="idf32")
            make_identity(nc, idf32[:])
            psT = bigp.tile([P, KC * R], F8, tag="ptbuf")
            for ntb in range(NNT):
                for u in range(4):
                    ptb = ps_tile(f"pa{(ntb * 4 + u) % 4}", dtype=F32)
                    for mt in range(NMT):
                        nc.tensor.transpose(
                            ptb[:, mt * P:(mt + 1) * P],
                            y_all[:, mt * B + ntb * TN + u * P:
                                  mt * B + ntb * TN + (u + 1) * P],
                            idf32[:])
                    nc.scalar.activation(
                        psT[:, (ntb * 4 + u) * R:(ntb * 4 + u + 1) * R],
                        ptb[:], ACT.Copy)

            # ============ GEMM1: Aᵀ chunks = (P²)ᵀ, consume C2, cast fp8 ====
            c2cols = colp.tile([P, KC], F32, tag="c2cols")
            c3cols = colp.tile([P, KC], F32, tag="c3cols")
            aT = bigp.tile([P, KC * R], F8, tag="atbuf")

            def lhs_strip_load(cbi, phase):
                ls = iop.tile([P, KC * P], F8, tag="lhs_strip",
                              name=f"ls{phase}_{cbi}")
                nc.sync.dma_start(
                    ls[:].rearrange("p (c m) -> p c m", c=KC),
                    bass.AP(pfull, cbi * P, [[B, P], [P * B, KC], [1, P]]))
                return ls

            def pen_t_tile(cbi):
                # penᵀ chunk [g-block, i]: relu(s_i − s_g)
                pt = workp.tile([P, TN], F16, tag="w3", name=f"pent{cbi}")
                nc.scalar.activation(pt[:], smy_b[:], ACT.Relu,
                                     bias=sfc[:, cbi:cbi + 1], scale=1.0)
                return pt

            for cb in range(KC):
                ls = lhs_strip_load(cb, 0)
                pa = ps_tile(f"pa{cb % 4}")
                for kb in range(KC2):
                    nc.tensor.matmul(
                        pa[:],
                        ls[:, 2 * kb * P:(2 * kb + 2) * P]
                        .rearrange("p (k m) -> p k m", k=2),
                        psT[:, 2 * kb * R:(2 * kb + 2) * R]
                        .rearrange("p (k n) -> p k n", k=2),
                        start=(kb == 0), stop=(kb == KC2 - 1), perf_mode=DRM)
                # diagonal corrections:
                # Aᵀ = (Q8²)ᵀ + d_g'·Q8ᵀ + Q8ᵀ·d_i + dsel·diag(d²)
                qt = psT[:, cb * R:(cb + 1) * R]
                nc.vector.scalar_tensor_tensor(
                    pa[:], qt, d_cols[:, cb:cb + 1], pa[:],
                    op0=OP.mult, op1=OP.add)
                tdi = workp.tile([P, TN], F32, tag="sqscr", name="tdi")
                nc.vector.tensor_tensor(tdi[:], qt, d_strip[:], OP.mult)
                nc.vector.tensor_tensor(pa[:], pa[:], tdi[:], OP.add)
                dgo = (cb % NMT) * P
                nc.vector.scalar_tensor_tensor(
                    pa[:, dgo:dgo + P], ident16[:], d2g[:, cb:cb + 1],
                    pa[:, dgo:dgo + P], op0=OP.mult, op1=OP.add)
                pent = pen_t_tile(cb)
                junk = workp.tile([P, TN], F16, tag="w1", name="junkA")
                nc.vector.scalar_tensor_tensor(
                    junk[:], pa[:], 1.0, pent[:], op0=OP.mult, op1=OP.mult,
                    accum_out=c2cols[:, cb:cb + 1])
                nc.scalar.activation(aT[:, cb * R:(cb + 1) * R], pa[:],
                                     ACT.Copy)

            # ============ GEMM2: Uᵀ chunks = (A·P)ᵀ, consume C3 =============
            for cb in range(KC):
                ls = lhs_strip_load(cb, 1)
                pa = ps_tile(f"pa{4 + cb % 4}")
                for kb in range(KC2):
                    nc.tensor.matmul(
                        pa[:],
                        ls[:, 2 * kb * P:(2 * kb + 2) * P]
                        .rearrange("p (k m) -> p k m", k=2),
                        aT[:, 2 * kb * R:(2 * kb + 2) * R]
                        .rearrange("p (k n) -> p k n", k=2),
                        start=(kb == 0), stop=(kb == KC2 - 1), perf_mode=DRM)
                # Uᵀ = (A·Q8)ᵀ + d_g'·Aᵀ
                nc.vector.scalar_tensor_tensor(
                    pa[:], aT[:, cb * R:(cb + 1) * R], d_cols[:, cb:cb + 1],
                    pa[:], op0=OP.mult, op1=OP.add)
                pent = pen_t_tile(cb)
                junk = workp.tile([P, TN], F16, tag="w1", name="junkU")
                nc.vector.scalar_tensor_tensor(
                    junk[:], pa[:], 1.0, pent[:], op0=OP.mult, op1=OP.mult,
                    accum_out=c3cols[:, cb:cb + 1])

            # ============ final reduction ==================================
            c2v = colp.tile([P, 1], F32, tag="c2v")
            c3v = colp.tile([P, 1], F32, tag="c3v")
            nc.vector.reduce_sum(c2v[:], c2cols[:], axis=AX.X)
            nc.vector.reduce_sum(c3v[:], c3cols[:], axis=AX.X)
            tot = colp.tile([P, 1], F32, tag="tot")
            nc.vector.tensor_scalar(tot[:], c2v[:], 0.5, None, OP.mult)
            nc.vector.tensor_tensor(tot[:], tot[:], c1v[:], OP.add)
            nc.vector.tensor_scalar(c3v[:], c3v[:], 1.0 / 3.0, None, OP.mult)
            nc.vector.tensor_tensor(tot[:], tot[:], c3v[:], OP.add)

            ones_c = constp.tile([P, 1], F32, tag="ones_c")
            nc.vector.memset(ones_c[:], 1.0)
            fin = ps_tile("pa0", shape=[1, 8])
            nc.tensor.matmul(fin[:, 0:1], tot[:], ones_c[:], start=True,
                             stop=True)
            lsb = colp.tile([1, 8], F32, tag="lsb")
            nc.vector.memset(lsb[:], 0.0)
            nc.scalar.activation(lsb[:, 0:1], fin[:, 0:1], ACT.Copy,
                                 scale=1.0 / float(B))
            nc.sync.dma_start(red_in[:, :], lsb[:])
            nc.gpsimd.collective_compute(
                "AllReduce", OP.add, replica_groups=rg,
                ins=[red_in.ap().opt()], outs=[red_out.ap().opt()])
            nc.sync.dma_start(loss_out[:, :], red_out[0:1, 0:1])

    nc.compile()
    return nc


def make_inputs(features, scores, labels, B, D, NC):
    """Build the per-core input maps from full inputs."""
    R = B // NC
    P = 128
    NMT = R // P
    NNT = B // R
    f = np.ascontiguousarray(features, dtype=np.float32)
    s = np.ascontiguousarray(scores, dtype=np.float32).reshape(B)
    lab = np.asarray(labels).astype(np.float32).reshape(B)
    ftf = np.ascontiguousarray(f.T)
    in_maps = []
    for c in range(NC):
        sh = slice(c * R, (c + 1) * R)
        onehot = np.zeros((1, NNT), dtype=np.float32)
        onehot[0, c] = 1.0
        in_maps.append({
            "ft2": np.ascontiguousarray(2.0 * f[sh].T).astype(ml_dtypes.bfloat16),
            "ftf": ftf.astype(ml_dtypes.bfloat16),
            "fsh": np.ascontiguousarray(f[sh]).astype(ml_dtypes.bfloat16),
            "srow": s.reshape(1, B),
            "srow16": s.reshape(1, B).astype(np.float16),
            "smyrow16": np.ascontiguousarray(s[sh]).reshape(1, R)
            .astype(np.float16),
            "scols": np.ascontiguousarray(s[sh].reshape(NMT, P).T),
            "lrow16": lab.reshape(1, B).astype(np.float16),
            "lcols": np.ascontiguousarray(lab[sh].reshape(NMT, P).T),
            "dsel": onehot,
        })
    return in_maps


_cached = {}


def kernel(features, scores, labels):
    B, D = features.shape
    NC = 8
    key = (B, D)
    if key not in _cached:
        _cached[key] = build_program(B=B, D=D, NC=NC)
    nc = _cached[key]
    from concourse.bass_utils import run_bass_kernel_spmd
    in_maps = make_inputs(features, scores, labels, B, D, NC)
    res = run_bass_kernel_spmd(nc, in_maps, core_ids=list(range(NC)))
    out = res.results[0]["loss"]
    return np.float32(out.reshape(())[()])
